# revision 54
# baseline (speedup 1.0000x reference)
"""Trainium2 Bass kernel for nn_LorentzLayer.

Math: the reference applies a per-cluster weighted Lorentz boost to T[b,c,:],
sums over clusters, then applies a second (inner) boost:

    out[b,a] = sum_{c,d} (B_inner @ (W_c * B_outer_c))[a,d] * T[b,c,d]

Both boosts compose into a single tiny matrix Mfull (400, 4) applied to
T flattened to (262144, 400):  out = Tf @ Mfull.

Device strategy (8 cores, pure batch data-parallel; kernel is DMA-bound so
input bytes are the lever, with accuracy budget rel-l2 < 2e-2):
  - Host computes Mfull in float64 (it only depends on the tiny inputs).
  - Row permutation is free (host reorders rows of T and Mfull together):
    the 256 K-rows with the smallest ||Mfull[j,:]|| ship as fp8 e4m3
    (2 chunks), the next 128 as bf16 (1 chunk), the top-norm 16 as bf16
    "rag" rows. Measured rel-l2 err 1.66e-2 on the reference inputs.
    fp8 chunks use a bf16 stationary (mixed-dtype matmul, supported on
    TRN2), so M carries no fp8 quantization error.
  - All input bytes for a subtile are fused into ONE (128, 17KB-row) DMA:
    [bf16 chunk | fp8 chunk0 | fp8 chunk1 | ragF], with the fp8 region
    addressed via AP.bitcast, issued as two column-halves, one per
    HWDGE ring, so both rings work each subtile at full 128-partition
    engine coverage (64-row partition splits measured ~60% slower -
    each covers only half the SDMA engines).
  - The 16 rag rows are folded 8x across partitions (ragF[r+16g, i] =
    rag[r, g*nb/8+i]) so their contribution is ONE 512-col matmul per
    subtile into a separate (32, nb/8) PSUM tile (vs. a K=16 matmul per
    512-col tile, which wasted a full PE pass on 4% of the data). The
    extra (32, B_core/8) output is recombined on host.
  - Per 512-col PSUM tile: 3 matmuls (bf16 K=128, 2x fp8 K=128)
    accumulate into a (4, 512) PSUM group; DVE copies to SBUF; outputs
    stored f32 with store base partitions rotating across SDMA engines.

Measured on trn2 (8 cores, axon): ~65-74 us/pass (session-dependent;
baseline hi/lo fp32-exact kernel: 166 us). DMA floor ~56 us, PE ~49 us.
"""

import numpy as np
import ml_dtypes

BF16 = ml_dtypes.bfloat16
F8 = ml_dtypes.float8_e4m3   # TRN fp8e4 flavor (max 240)

BATCH = 262144
CLUSTER = 100
KDIM = 4 * CLUSTER  # 400
NCORES = 8
B_CORE = BATCH // NCORES  # 32768
NB = 4096    # batch subtile (columns per DMA)
NPS = 512    # psum tile free size
RAG = 16     # ragged bf16 rows (KDIM - 3*128)

# Tunables (module-level so probes can flip them)
N8CH = 2            # number of 128-row fp8 chunks (0..2)
STAT8_BF16 = True   # bf16 stationary for fp8 chunks (mixed-dtype matmul)
BIG_SPLIT = "colsplit"  # 'colsplit' | 'whole' | 'half'
OUT_BF16 = False    # store output as bf16
OUT_SWDGE = False   # out stores via SWDGE ring
OUT_ROTATE = True   # rotate out/psum base partition across SDMA engines
RAG_MODE = "fold"  # 'fold' (in-DMA, 1 matmul/subtile) | 'preload' | 'subtile'
COPY_SPLIT = False  # alternate PSUM->SBUF copies between DVE and ACT
PS_CAP = 6          # fold mode: j-loop psum banks (psr gets 8-PS_CAP)
BUFS_IN = 4
BUFS_PS = 8
BUFS_OUT = 4


def _build_nc(b_core: int, nb: int, repeat: int = 1, mode: str = "full",
              n8ch: int = None, bufs_in: int = None, bufs_ps: int = None,
              bufs_out: int = None, out_bf16: bool = None,
              out_rotate: bool = None, rag_mode: str = None,
              out_swdge: bool = None, big_split: str = None,
              skip_rag: bool = False, skip_out: bool = False,
              stat8_bf16: bool = None, skip_f8mm: bool = False,
              copy_split: bool = None):
    """mode: 'full' | 'dma' (loads only) | 'compute' (no big loads).
    repeat>1 wraps the pass in a device-side For_i loop (timing harness)."""
    import concourse.bacc as bacc
    import concourse.tile as tile
    import concourse.mybir as mybir

    if n8ch is None:
        n8ch = N8CH
    if bufs_in is None:
        bufs_in = BUFS_IN
    if bufs_ps is None:
        bufs_ps = BUFS_PS
    if bufs_out is None:
        bufs_out = BUFS_OUT
    if out_bf16 is None:
        out_bf16 = OUT_BF16
    if out_rotate is None:
        out_rotate = OUT_ROTATE
    if rag_mode is None:
        rag_mode = RAG_MODE
    if out_swdge is None:
        out_swdge = OUT_SWDGE
    if big_split is None:
        big_split = BIG_SPLIT
    if stat8_bf16 is None:
        stat8_bf16 = STAT8_BF16
    if copy_split is None:
        copy_split = COPY_SPLIT

    nch16 = 3 - n8ch  # 128-row bf16 chunks

    bf16 = mybir.dt.bfloat16
    f8 = mybir.dt.float8e4
    f32 = mybir.dt.float32
    out_dt = bf16 if out_bf16 else f32

    nc = bacc.Bacc("TRN2", target_bir_lowering=False, debug=False,
                   num_devices=NCORES)

    # fused input: per subtile, per partition row the bytes are
    # [chunk0 bf16 | .. | chunk_{nch16-1} bf16 | fp8 chunks], declared as a
    # bf16 tensor of (6-n8ch)*nb/2 cols per subtile; the fp8 region is
    # reached through AP.bitcast so one uniform (128, 10KB-row) DMA feeds
    # all chunks.
    rf = nb // 8                                # ragF cols (fold mode)
    bcols = (6 - n8ch) * nb // 2                # bf16 cols per subtile
    if rag_mode == "fold":
        bcols += rf
    big = nc.dram_tensor("big", (128, bcols * (b_core // nb)), bf16,
                         kind="ExternalInput")
    rag = None
    if rag_mode == "preload":
        # rag rows folded 4x across partitions: ragf[16g+r, w] =
        # rag_row[r, g*(b_core//4) + w]; loaded once per pass
        rag = nc.dram_tensor("rag", (4 * RAG, b_core // 4), bf16,
                             kind="ExternalInput")
    elif rag_mode == "subtile":
        rag = nc.dram_tensor("rag", (RAG, b_core), bf16,
                             kind="ExternalInput")
    stat = nc.dram_tensor("stat", (128, 48), bf16, kind="ExternalInput")
    stat8 = None
    if n8ch:
        stat8 = nc.dram_tensor("stat8", (128, 4 * n8ch),
                               bf16 if stat8_bf16 else f8,
                               kind="ExternalInput")
    outT = nc.dram_tensor("outT", (4, b_core), out_dt, kind="ExternalOutput")
    outR = None
    if rag_mode == "fold":
        outR = nc.dram_tensor("outR", (32, b_core // 8), out_dt,
                              kind="ExternalOutput")

    n_sub = b_core // nb
    n_ps = nb // NPS
    gcols = b_core // 4
    assert rag_mode != "preload" or gcols % NPS == 0
    do_dma = mode in ("full", "dma")
    do_compute = mode in ("full", "compute")

    with tile.TileContext(nc) as tc:
        with (
            tc.tile_pool(name="statp", bufs=1) as statpool,
            tc.tile_pool(name="inp", bufs=bufs_in) as inpool,
            tc.tile_pool(name="ragp", bufs=2) as ragpool,
            tc.tile_pool(name="outp", bufs=bufs_out) as outpool,
            tc.tile_pool(name="ps",
                         bufs=(min(bufs_ps, PS_CAP) if rag_mode == "fold"
                               else bufs_ps), space="PSUM") as pspool,
            tc.tile_pool(name="psr", bufs=8 - PS_CAP,
                         space="PSUM") as psrpool,
        ):
            stat_sb = statpool.tile([128, 48], bf16)
            nc.sync.dma_start(out=stat_sb[:, :], in_=stat[:, :])
            stat8_sb = None
            if n8ch:
                stat8_sb = statpool.tile([128, 4 * n8ch],
                                         bf16 if stat8_bf16 else f8)
                nc.scalar.dma_start(out=stat8_sb[:, :], in_=stat8[:, :])

            if not do_dma:
                dummy_in = statpool.tile([128, bcols], bf16)
                nc.gpsimd.memset(dummy_in[:, :], 0)
                dummy_rag = None
                if rag_mode != "fold":
                    dummy_rag = statpool.tile(
                        [128, gcols if rag_mode == "preload" else nb], bf16)
                    nc.gpsimd.memset(dummy_rag[:, :], 0)

            def aux_eng(s):
                return nc.scalar if s % 2 == 0 else nc.sync

            def pass_body():
                prt = None
                if rag_mode == "preload":
                    if do_dma:
                        prt = ragpool.tile([128, gcols], bf16)
                        if not skip_rag:
                            for g in range(4):
                                eng = nc.sync if g % 2 == 0 else nc.scalar
                                eng.dma_start(
                                    out=prt[32 * g:32 * g + RAG, :],
                                    in_=rag[RAG * g:RAG * (g + 1), :])
                    else:
                        prt = dummy_rag
                for s in range(n_sub):
                    q = (0, 64, 32, 96)[s % 4]
                    if rag_mode == "fold":
                        rt = None
                    elif rag_mode == "preload":
                        rt = prt
                    elif do_dma:
                        rt = ragpool.tile([128, nb], bf16)
                        if not skip_rag:
                            aux_eng(s).dma_start(
                                out=rt[q:q + RAG, :],
                                in_=rag[:, s * nb:(s + 1) * nb])
                    else:
                        rt = dummy_rag
                    beng = nc.sync if s % 2 == 0 else nc.scalar
                    oeng = nc.scalar if s % 2 == 0 else nc.sync
                    if do_dma:
                        bt = inpool.tile([128, bcols], bf16, tag="b16")
                        csl = slice(s * bcols, (s + 1) * bcols)
                        if big_split == "whole":
                            beng.dma_start(out=bt[:, :], in_=big[:, csl])
                        elif big_split == "colsplit":
                            # both rings work the same subtile: full
                            # 128-partition coverage, per-subtile balance
                            h = bcols // 2
                            nc.sync.dma_start(
                                out=bt[:, 0:h],
                                in_=big[:, s * bcols:s * bcols + h])
                            nc.scalar.dma_start(
                                out=bt[:, h:bcols],
                                in_=big[:, s * bcols + h:(s + 1) * bcols])
                        elif big_split == "half":
                            nc.sync.dma_start(out=bt[0:64, :],
                                              in_=big[0:64, csl])
                            nc.scalar.dma_start(out=bt[64:128, :],
                                                in_=big[64:128, csl])
                        else:
                            raise ValueError(big_split)
                    else:
                        bt = dummy_in
                    # out/psum base partition rotates (tile_position only
                    # allows multiples of 32) so the (4, nb) store doesn't
                    # pile onto the one engine serving partitions 0-7
                    q2 = (0, 64, 32, 96)[(s + 2) % 4] if out_rotate else 0
                    qr = (0, 64, 32, 96)[(s + 1) % 4] if out_rotate else 0
                    if rag_mode == "fold":
                        roff = (6 - n8ch) * nb // 2
                        otr = outpool.tile([128, rf], out_dt, tag="outr")
                        if do_compute:
                            psr = psrpool.tile([128, NPS], f32)
                            nc.tensor.matmul(psr[qr:qr + 32, 0:rf],
                                             stat_sb[:, 16:48],
                                             bt[:, roff:roff + rf],
                                             start=True, stop=True,
                                             tile_position=(0, qr))
                            nc.vector.tensor_copy(otr[qr:qr + 32, :],
                                                  psr[qr:qr + 32, 0:rf])
                        else:
                            nc.gpsimd.memset(otr[qr:qr + 32, 0:1], 0)
                        if do_dma and not skip_out:
                            reng = nc.gpsimd if out_swdge else aux_eng(s)
                            reng.dma_start(out=outR[:, s * rf:(s + 1) * rf],
                                           in_=otr[qr:qr + 32, :])
                    ot = outpool.tile([128, nb] if out_rotate else [4, nb],
                                      out_dt)
                    ots = ot[q2:q2 + 4, :]
                    if not do_compute:
                        nc.gpsimd.memset(ots[:, 0:1], 0)
                    if do_compute:
                        for j in range(n_ps):
                            ps = pspool.tile([128, NPS] if out_rotate
                                             else [4, NPS], f32)
                            pss = ps[q2:q2 + 4, :]
                            jsl = slice(j * NPS, (j + 1) * NPS)
                            do_rag = (rag_mode != "fold"
                                      and not (skip_rag and do_dma))
                            if do_rag and rag_mode == "preload":
                                c0 = s * nb + j * NPS
                                g, w0 = c0 // gcols, c0 % gcols
                                # PSUM start=True clears the whole bank
                                nc.tensor.matmul(
                                    pss[:, :],
                                    stat_sb[32 * g:32 * g + RAG, 12:16],
                                    rt[32 * g:32 * g + RAG, w0:w0 + NPS],
                                    start=True, stop=False,
                                    tile_position=(32 * g, q2))
                            elif do_rag:
                                nc.tensor.matmul(pss[:, :],
                                                 stat_sb[q:q + RAG, 12:16],
                                                 rt[q:q + RAG, jsl],
                                                 start=True, stop=False,
                                                 tile_position=(q, q2))
                            for k in range(nch16):
                                ksl = slice(k * nb + j * NPS,
                                            k * nb + (j + 1) * NPS)
                                nc.tensor.matmul(
                                    pss[:, :],
                                    stat_sb[:, 4 * k:4 * k + 4],
                                    bt[:, ksl],
                                    start=(k == 0 and not do_rag),
                                    stop=(k == nch16 - 1 and
                                          (not n8ch or skip_f8mm)),
                                    tile_position=(0, q2))
                            for k in range(0 if skip_f8mm else n8ch):
                                off = nch16 * nb + k * nb // 2
                                ksl = slice(off + j * NPS // 2,
                                            off + (j + 1) * NPS // 2)
                                nc.tensor.matmul(
                                    pss[:, :],
                                    stat8_sb[:, 4 * k:4 * k + 4],
                                    bt[:, ksl].bitcast(f8),
                                    start=False, stop=(k == n8ch - 1),
                                    tile_position=(0, q2))
                            if copy_split and j % 2:
                                nc.scalar.copy(out=ots[:, jsl],
                                               in_=pss[:, :])
                            else:
                                nc.vector.tensor_copy(ots[:, jsl],
                                                      pss[:, :])
                    if do_dma and not skip_out:
                        seng = nc.gpsimd if out_swdge else oeng
                        seng.dma_start(
                            out=outT[:, s * nb:(s + 1) * nb], in_=ots[:, :])

            if repeat > 1:
                with tc.For_i(0, repeat, 1,
                              hint_engines=(mybir.EngineType.PE,
                                            mybir.EngineType.DVE,
                                            mybir.EngineType.SP,
                                            mybir.EngineType.Activation)):
                    pass_body()
            else:
                pass_body()

    nc.compile()
    return nc


def _boost_mats(boosts: np.ndarray, K_mats: np.ndarray) -> np.ndarray:
    """boosts (C,3) -> Lorentz boost matrices (C,4,4), float64."""
    b = boosts.astype(np.float64)
    K = K_mats.astype(np.float64)
    mag = np.sqrt((b * b).sum(axis=1, keepdims=True))        # (C,1)
    n = b / mag                                              # (C,3)
    g = 1.0 / np.sqrt(1.0 - mag * mag)                       # (C,1)
    nK = np.einsum('cj,jad->cad', n, K)                      # (C,4,4)
    nK2 = np.einsum('cab,cbd->cad', nK, nK)                  # (C,4,4)
    B = (np.eye(4)[None]
         - (g * mag)[..., None] * nK
         + (g - 1.0)[..., None] * nK2)
    return B


def _mfull(Bo, Bi, W, K_mats) -> np.ndarray:
    """Composite matrix Mfull (400, 4): out[b,a] = sum_j Tf[b,j] Mfull[j,a]."""
    Bc = _boost_mats(Bo, K_mats)                  # (C,4,4)
    B2 = _boost_mats(Bi, K_mats)[0]               # (4,4)
    comp = np.einsum('ad,cde->cae', B2, Bc)       # (C,4,4) = B2 @ Bc
    comp = comp * W.astype(np.float64)[:, None]   # weight per cluster
    # Mfull[c*4+d, a] = comp[c, a, d]
    return np.ascontiguousarray(comp.transpose(0, 2, 1).reshape(KDIM, 4))


def _row_split(Mfull64: np.ndarray, n8ch: int):
    """Row assignment: the 128*n8ch smallest-||M|| rows go fp8; of the
    rest, the first 128*(3-n8ch) go to bf16 chunks, the last 16 to rag."""
    order = np.argsort(np.linalg.norm(Mfull64, axis=1), kind="stable")
    idx8 = order[:128 * n8ch]
    rest = np.sort(order[128 * n8ch:])
    return idx8, rest[:128 * (3 - n8ch)], rest[128 * (3 - n8ch):]


def _pack_stationary(Mfull64: np.ndarray, n8ch: int,
                     stat8_bf16: bool = None):
    """-> stat (128, 16) bf16, stat8 (128, 4*n8ch)."""
    if stat8_bf16 is None:
        stat8_bf16 = STAT8_BF16
    idx8, idx16, idxrag = _row_split(Mfull64, n8ch)
    Mb = Mfull64.astype(np.float32).astype(BF16)  # (400, 4)
    stat = np.zeros((128, 48), dtype=BF16)
    for k in range(3 - n8ch):
        stat[:, 4 * k:4 * k + 4] = Mb[idx16[k * 128:(k + 1) * 128]]
    for qi in range(4):
        stat[32 * qi:32 * qi + RAG, 12:16] = Mb[idxrag]
    # fold-mode rag stationary: S[r+16g, 16+4g+a] = M_rag[r, a]
    for g in range(8):
        stat[16 * g:16 * g + RAG, 16 + 4 * g:16 + 4 * g + 4] = Mb[idxrag]
    sdt = BF16 if stat8_bf16 else F8
    stat8 = np.zeros((128, max(4 * n8ch, 4)), dtype=sdt)
    if n8ch:
        M8 = Mb if stat8_bf16 else Mfull64.astype(np.float32).astype(F8)
        for k in range(n8ch):
            stat8[:, 4 * k:4 * k + 4] = M8[idx8[k * 128:(k + 1) * 128]]
    return stat, stat8


def _pack_core(Tt: np.ndarray, Mfull64: np.ndarray, b_core: int, nb: int,
               n8ch: int = None, rag_mode: str = None):
    """Tt (400, b_core) f32 -> {'big'[, 'rag']} device layouts."""
    if n8ch is None:
        n8ch = N8CH
    if rag_mode is None:
        rag_mode = RAG_MODE
    idx8, idx16, idxrag = _row_split(Mfull64, n8ch)
    nch16 = 3 - n8ch
    n_sub = b_core // nb
    out = {}
    b16 = Tt[idx16].astype(BF16)                 # (nch16*128, b_core)
    # (128, n_sub, nch16, nb) -> per-subtile bf16 chunk bytes
    p16 = np.ascontiguousarray(
        b16.reshape(nch16, 128, n_sub, nb).transpose(1, 2, 0, 3))
    pieces = [p16.view(np.uint8).reshape(128, n_sub, nch16 * nb * 2)]
    if n8ch:
        b8 = Tt[idx8].astype(F8)                 # (n8ch*128, b_core)
        p8 = np.ascontiguousarray(
            b8.reshape(n8ch, 128, n_sub, nb).transpose(1, 2, 0, 3))
        pieces.append(p8.view(np.uint8).reshape(128, n_sub, n8ch * nb))
    ragT = Tt[idxrag].astype(BF16)               # (16, b_core)
    if rag_mode == "fold":
        rf = nb // 8
        # ragF[r+16g, s, i] = ragT[r, s*nb + g*rf + i]
        rF = np.ascontiguousarray(
            ragT.reshape(RAG, n_sub, 8, rf).transpose(2, 0, 1, 3)
        ).reshape(128, n_sub, RAG // 16 * rf)
        pieces.append(rF.view(np.uint8).reshape(128, n_sub, rf * 2))
    fused = np.concatenate(pieces, axis=2)
    out["big"] = np.ascontiguousarray(fused).view(BF16).reshape(128, -1)
    if rag_mode == "preload":
        out["rag"] = np.ascontiguousarray(
            ragT.reshape(RAG, 4, b_core // 4).transpose(1, 0, 2)
        ).reshape(4 * RAG, b_core // 4)
    elif rag_mode == "subtile":
        out["rag"] = np.ascontiguousarray(ragT)
    return out


_NC_CACHE = {}


def _get_nc():
    key = (B_CORE, NB, N8CH, BIG_SPLIT, OUT_BF16, OUT_ROTATE,
           RAG_MODE, OUT_SWDGE, BUFS_IN, BUFS_PS, BUFS_OUT)
    if key not in _NC_CACHE:
        _NC_CACHE[key] = _build_nc(B_CORE, NB)
    return _NC_CACHE[key]


def _selftest_small():
    """CoreSim structural/numeric check at reduced size (no hardware)."""
    from concourse.bass_interp import CoreSim
    b_core_t, nb_t = 2048, 512
    rng = np.random.default_rng(0)
    Tt = rng.standard_normal((KDIM, b_core_t)).astype(np.float32)
    Mfull = rng.standard_normal((KDIM, 4)).astype(np.float64) * 0.3
    stat, stat8 = _pack_stationary(Mfull, N8CH)
    packs = _pack_core(Tt, Mfull, b_core_t, nb_t)
    nc = _build_nc(b_core_t, nb_t)
    sim = CoreSim(nc, require_finite=True, require_nnan=True)
    sim.tensor("stat")[:] = stat
    if N8CH:
        sim.tensor("stat8")[:] = stat8[:, :4 * N8CH]
    for k, v in packs.items():
        sim.tensor(k)[:] = v
    sim.simulate(check_with_hw=False)
    oR = (np.asarray(sim.tensor("outR")) if RAG_MODE == "fold" else None)
    got = _assemble_core(np.asarray(sim.tensor("outT")), oR, b_core_t, nb_t)
    # emulate quantization for the expected value
    idx8, idx16, idxrag = _row_split(Mfull, N8CH)
    q = Tt.astype(BF16).astype(np.float64)
    Mq = Mfull.astype(np.float32).astype(BF16).astype(np.float64)
    if N8CH:
        q[idx8] = Tt[idx8].astype(F8).astype(np.float64)
        if not STAT8_BF16:
            Mq[idx8] = Mfull[idx8].astype(np.float32).astype(F8).astype(np.float64)
    want = q.T @ Mq
    rel = np.linalg.norm(got - want) / np.linalg.norm(want)
    assert rel < 1e-4, rel
    return rel


def _assemble_core(o4: np.ndarray, oR: np.ndarray | None,
                   b_core: int, nb: int) -> np.ndarray:
    """o4 (4, b_core) [+ oR (32, b_core//8) fold part] -> (b_core, 4)."""
    out = o4.astype(np.float32).T.copy()
    if oR is not None:
        n_sub = b_core // nb
        rf = nb // 8
        X = oR.astype(np.float32).reshape(8, 4, n_sub, rf)
        out += X.transpose(2, 0, 3, 1).reshape(b_core, 4)
    return out


def prepare_in_maps(T, Bo, Bi, W, K_mats, nb=None):
    nbv = nb if nb is not None else NB
    T = np.asarray(T, dtype=np.float32)
    Mfull = _mfull(np.asarray(Bo), np.asarray(Bi),
                   np.asarray(W), np.asarray(K_mats))
    stat, stat8 = _pack_stationary(Mfull, N8CH)
    Tf = T.reshape(BATCH, KDIM)
    in_maps = []
    for c in range(NCORES):
        Tt = np.ascontiguousarray(Tf[c * B_CORE:(c + 1) * B_CORE].T)
        m = _pack_core(Tt, Mfull, B_CORE, nbv)
        m["stat"] = stat
        if N8CH:
            m["stat8"] = stat8[:, :4 * N8CH]
        in_maps.append(m)
    return in_maps


# Set by test harnesses to profile the run; kernel() stores the spmd results
# object (exec_time_ns etc.) in LAST_RESULTS when TRACE is on.
TRACE = False
TRACE_KWARGS = {}
LAST_RESULTS = None


def kernel(T, Bo, Bi, W, K_mats):
    from concourse.bass_utils import run_bass_kernel_spmd

    in_maps = prepare_in_maps(T, Bo, Bi, W, K_mats)
    nc = _get_nc()
    res = run_bass_kernel_spmd(nc, in_maps, core_ids=list(range(NCORES)),
                               trace=TRACE, **TRACE_KWARGS)
    if TRACE:
        global LAST_RESULTS
        LAST_RESULTS = res

    out = np.empty((BATCH, 4), dtype=np.float32)
    for c in range(NCORES):
        o4 = np.asarray(res.results[c]["outT"])                    # (4, Bc)
        oR = (np.asarray(res.results[c]["outR"])
              if RAG_MODE == "fold" else None)
        out[c * B_CORE:(c + 1) * B_CORE] = _assemble_core(
            o4, oR, B_CORE, NB)
    return out.reshape(BATCH, 1, 4)
